# revision 38
# baseline (speedup 1.0000x reference)
"""MoE feed-forward (top-1 routing) on 8 TRN2 NeuronCores.

Sharding: expert-parallel with a tensor-parallel split of the hidden dim.
Core c handles expert e = c // 2 and hidden half hh = c % 2 (H=3072 -> 1536
per core).  The host computes the (tiny) gate + argmax routing, gathers each
expert's tokens into a fixed-capacity feature-major buffer (the dispatch /
all-to-all step of the sharding), and each core runs

    y_half = GELU(x_e @ W1[e][:, half] + b1[e][half]) @ W2[e][half, :] (+ b2[e])

entirely on device (f32r matmuls on the PE + Gelu on the ACT engine).  The
host combine adds the two hidden-half partial outputs of each expert pair and
scatters rows back to token positions.

Toolchain note: this walrus build accepts at most ONE sync-wait per
instruction.  The kernel is structured so Tile never needs more: <=8 DMAs
(no HW-queue sem reuse), "observer" ops that let PE/ACT see input DMAs once,
and a TileContext subclass that splits the final drain's waits.
"""

import sys

sys.path.insert(0, "/opt/trn_rl_repo")

import numpy as np

import concourse.bass as bass
import concourse.mybir as mybir
import concourse.tile as tile
from concourse import bass_utils
from concourse.vector_clock import ScopedClock

B, T, E, H, NEXP = 2, 1024, 768, 3072, 4
NCORES = 8
HH = H // 2          # hidden half per core: 1536
KE = E // 128        # 6   k-chunks over E
KH = HH // 128       # 12  k-chunks over HH
NSPLIT = 320         # matmul moving free-dim tile (>=256 keeps f32r at 1 cyc/row)

_MAXW = 1  # walrus allows a single sync-wait per instruction


class _SplitDrainTC(tile.TileContext):
    """TileContext whose final drain splits its sem waits across single-wait
    sync-engine event-sem instructions."""

    def _drain_and_barrier(self, tick_clock, wait_clock):
        carrier = self.nc.sync.nop(nofuse=True)
        wait_clock.add_sem_waits(
            carrier.ins, ScopedClock({None: tick_clock.global_clock})
        )
        waits = list(carrier.ins.sync_info.on_wait or [])
        if len(waits) > _MAXW:
            handles = {h.name: h for h in self.sems.allocated().values()}
            carrier.ins.sync_info.on_wait = waits[:_MAXW]
            for w in waits[_MAXW:]:
                self.nc.sync.wait_ge(handles[w.ant_name], w.wait_value)
        self.nc.sync.drain()
        self.nc.all_engine_barrier()
        popped = self.nc._tile_sem_poison_stack.pop()
        assert popped is self._sem_poison
        # The sem clear runs on the sync engine after the barrier; every other
        # engine's stream has already ended and the runtime serializes NEFF
        # executions, so the closing all-engine barrier is dead time and is
        # omitted.
        self.nc.clear_and_free_semaphores(list(self.sems.allocated().values()))


_prog_cache: dict[int, bass.Bass] = {}
_runner_cache: dict[int, object] = {}


class _Runner:
    """Compile once, execute many: replicates bass2jax.run_bass_via_pjrt but
    caches the jitted shard_map executable so repeat kernel() calls skip
    retracing, and exposes device-resident execution for timing."""

    def __init__(self, nc: bass.Bass):
        import jax
        from jax.sharding import Mesh, PartitionSpec, NamedSharding
        from jax.experimental.shard_map import shard_map
        from concourse import bass2jax

        bass2jax.install_neuronx_cc_hook()
        self.jax = jax
        partition_name = (
            nc.partition_id_tensor.name if nc.partition_id_tensor else None
        )
        in_names, out_names, out_avals, zero_outs = [], [], [], []
        for alloc in nc.m.functions[0].allocations:
            if not isinstance(alloc, mybir.MemoryLocationSet):
                continue
            name = alloc.memorylocations[0].name
            if alloc.kind == "ExternalInput":
                if name != partition_name:
                    in_names.append(name)
            elif alloc.kind == "ExternalOutput":
                shape = tuple(alloc.tensor_shape)
                dtype = mybir.dt.np(alloc.dtype)
                out_names.append(name)
                out_avals.append(jax.core.ShapedArray(shape, dtype))
                zero_outs.append(np.zeros(shape, dtype))
        self.in_names = list(in_names)
        self.out_names = out_names
        self.out_avals = out_avals
        self.zero_outs = zero_outs
        n_params = len(in_names)
        self.n_params = n_params
        all_in_names = list(in_names) + list(out_names)
        if partition_name is not None:
            all_in_names.append(partition_name)

        def _body(*args):
            operands = list(args)
            if partition_name is not None:
                operands.append(bass2jax.partition_id_tensor())
            outs = bass2jax._bass_exec_p.bind(
                *operands,
                out_avals=tuple(out_avals),
                in_names=tuple(all_in_names),
                out_names=tuple(out_names),
                lowering_input_output_aliases=(),
                sim_require_finite=True,
                sim_require_nnan=True,
                nc=nc,
            )
            return tuple(outs)

        devices = jax.devices()[:NCORES]
        self.mesh = Mesh(np.asarray(devices), ("core",))
        self.pspec = PartitionSpec("core")
        self.sharding = NamedSharding(self.mesh, self.pspec)
        n_outs = len(out_names)
        donate = tuple(range(n_params, n_params + n_outs))
        self.sharded = jax.jit(
            shard_map(
                _body,
                mesh=self.mesh,
                in_specs=(self.pspec,) * (n_params + n_outs),
                out_specs=(self.pspec,) * n_outs,
                check_rep=False,
            ),
            donate_argnums=donate,
            keep_unused=True,
        )

    def concat_inputs(self, in_maps):
        return [
            np.concatenate([np.asarray(m[name]) for m in in_maps], axis=0)
            for name in self.in_names
        ]

    def concat_zeros(self):
        return [
            np.zeros((NCORES * z.shape[0], *z.shape[1:]), z.dtype)
            for z in self.zero_outs
        ]

    def __call__(self, in_maps):
        out_arrs = self.sharded(*self.concat_inputs(in_maps), *self.concat_zeros())
        results = []
        for c in range(NCORES):
            results.append(
                {
                    name: np.asarray(out_arrs[i]).reshape(
                        NCORES, *self.out_avals[i].shape
                    )[c]
                    for i, name in enumerate(self.out_names)
                }
            )
        return results


def _get_runner(C: int) -> _Runner:
    r = _runner_cache.get(C)
    if r is None:
        nc = _prog_cache.get(C)
        if nc is None:
            nc = _build_program(C)
            _prog_cache[C] = nc
        r = _Runner(nc)
        _runner_cache[C] = r
    return r


def _build_program(C: int) -> bass.Bass:
    """One SPMD program: per-core expert-half MLP over C capacity tokens."""
    f32 = mybir.dt.float32
    f32r = mybir.dt.float32r
    nc = bass.Bass("TRN2", target_bir_lowering=False, num_devices=NCORES)

    xg = nc.dram_tensor("xg", [E, C], f32r, kind="ExternalInput")
    w1 = nc.dram_tensor("w1", [E, HH], f32r, kind="ExternalInput")
    w2 = nc.dram_tensor("w2", [HH, E], f32r, kind="ExternalInput")
    bias = nc.dram_tensor("bias", [HH + E], f32, kind="ExternalInput")  # b1|b2
    y = nc.dram_tensor("y", [E, C], f32, kind="ExternalOutput")

    ncol = C // NSPLIT  # moving-dim tiles per row block

    with _SplitDrainTC(nc) as tc:
        with (
            tc.tile_pool(name="wpool", bufs=1) as wp,
            tc.tile_pool(name="ps", bufs=7, space="PSUM") as ps,
            tc.tile_pool(name="psW", bufs=1, space="PSUM") as psw,
        ):
            xgs = wp.tile([128, KE, C], f32r, tag="xg")
            w1s = wp.tile([128, KE, HH], f32r, tag="w1")
            w2s = wp.tile([128, KH, E], f32r, tag="w2")
            bs = wp.tile([128, KH + KE], f32, tag="bias")
            xgv = xg.ap().rearrange("(k p) c -> p k c", p=128)
            w1v = w1.ap().rearrange("(k p) h -> p k h", p=128)
            w2v = w2.ap().rearrange("(k p) e -> p k e", p=128)
            # Input DMAs: bias + xg first, W1 in per-k chunks (so the first
            # wave of phase A can start as each chunk lands), then W2
            # (streams under phase A).  10 input + 6 output DMAs = two full
            # rounds of the 8 HWDGE lanes.
            nc.sync.dma_start(out=bs[:], in_=bias.ap().rearrange("(m p) -> p m", p=128))
            nc.sync.dma_start(out=xgs[:], in_=xgv[:])
            for k in range(KE):
                nc.sync.dma_start(out=w1s[:, k, :], in_=w1v[:, k, :])
            nc.sync.dma_start(out=w2s[:, : KH // 2, :], in_=w2v[:, : KH // 2, :])
            nc.sync.dma_start(out=w2s[:, KH // 2 :, :], in_=w2v[:, KH // 2 :, :])

            hs = wp.tile([128, KH, C], f32r, tag="h")
            ys = wp.tile([128, KE, C], f32, tag="y")

            # This walrus build allows one sync-wait per instruction, so
            # "observer" ops let each engine see an input DMA exactly once;
            # Tile's per-engine clock then elides those waits elsewhere.
            warm = psw.tile([2, 32], f32, tag="warm")
            obs_i = [0]

            def pe_obs(src):
                i = obs_i[0]
                obs_i[0] += 1
                nc.tensor.matmul(
                    warm[:, 2 * i : 2 * i + 2], src, src, start=True, stop=True
                )

            pe_obs(xgs[:, 0, :2])
            scratch = wp.tile([128, KE + 3], f32, tag="actwarm")
            nc.scalar.activation(
                scratch[:, 0:1], bs[:, 0:1], mybir.ActivationFunctionType.Copy
            )
            nc.vector.tensor_copy(scratch[:, KE + 2 : KE + 3], bs[:, 0:1])

            # Phase A: h = gelu(x @ W1h + b1h), feature-major [HH, C].
            # Wave 1 = first 6 (m, n) groups run k-outer across 6 psum banks,
            # paced by the w1 chunk DMAs, so the PE starts ~7.5us in instead
            # of waiting for the whole W1 load.
            wave1 = [(m, n) for m in range(3) for n in range(ncol)] + [(3, 0)]
            accs = {}
            for g in wave1:
                acc = ps.tile([128, NSPLIT], f32, tag="acc")
                accs[g] = acc
            # No per-chunk observers: the first wave-1 matmul of each k-chunk
            # carries the w1_k DMA wait itself (its only unobserved dep).
            for k in range(KE):
                for m, n in wave1:
                    nc.tensor.matmul(
                        accs[(m, n)][:],
                        w1s[:, k, m * 128 : (m + 1) * 128],
                        xgs[:, k, n * NSPLIT : (n + 1) * NSPLIT],
                        start=(k == 0),
                        stop=(k == KE - 1),
                    )
            for m, n in wave1:
                nc.scalar.activation(
                    hs[:, m, n * NSPLIT : (n + 1) * NSPLIT],
                    accs[(m, n)][:],
                    mybir.ActivationFunctionType.Gelu,
                    bias=bs[:, m : m + 1],
                )
            rest = [
                (m, n)
                for m in range(3, KH)
                for n in range(ncol)
                if (m, n) not in wave1
            ]
            for m, n in rest:
                if True:
                    acc = ps.tile([128, NSPLIT], f32, tag="acc")
                    for k in range(KE):
                        nc.tensor.matmul(
                            acc[:],
                            w1s[:, k, m * 128 : (m + 1) * 128],
                            xgs[:, k, n * NSPLIT : (n + 1) * NSPLIT],
                            start=(k == 0),
                            stop=(k == KE - 1),
                        )
                    nc.scalar.activation(
                        hs[:, m, n * NSPLIT : (n + 1) * NSPLIT],
                        acc[:],
                        mybir.ActivationFunctionType.Gelu,
                        bias=bs[:, m : m + 1],
                    )

            # w2 observers between the phases: w2's DMA streams under phase A,
            # so gating only phase B on it costs nothing.  ACT observers of
            # one w1 chunk per HWDGE lane let the per-m output DMAs skip
            # their lane-FIFO waits.
            pe_obs(w2s[:, 0, :2])
            pe_obs(w2s[:, KH // 2, :2])
            for k in range(KE):
                nc.scalar.activation(
                    scratch[:, k + 1 : k + 2],
                    w1s[:, k, 0:1],
                    mybir.ActivationFunctionType.Copy,
                )
            nc.scalar.activation(
                scratch[:, KE + 1 : KE + 2],
                xgs[:, 0, 0:1],
                mybir.ActivationFunctionType.Copy,
            )

            # Phase B: y = h @ W2h + b2, feature-major [E, C]; per-m output
            # DMAs issued from the ACT engine's HWDGE (producer engine, so no
            # cross-engine data wait).
            yv = y.ap().rearrange("(m p) c -> p m c", p=128)
            for m in range(KE):
                for n in range(ncol):
                    acc = ps.tile([128, NSPLIT], f32, tag="acc")
                    for k in range(KH):
                        nc.tensor.matmul(
                            acc[:],
                            w2s[:, k, m * 128 : (m + 1) * 128],
                            hs[:, k, n * NSPLIT : (n + 1) * NSPLIT],
                            start=(k == 0),
                            stop=(k == KH - 1),
                        )
                    nc.vector.tensor_scalar_add(
                        ys[:, m, n * NSPLIT : (n + 1) * NSPLIT],
                        acc[:],
                        bs[:, KH + m : KH + m + 1],
                    )
                nc.scalar.dma_start(out=yv[:, m, :], in_=ys[:, m, :])

    return nc


def _build_null_program(C: int) -> bass.Bass:
    """Same I/O signature, near-zero device work — timing baseline only."""
    f32 = mybir.dt.float32
    f32r = mybir.dt.float32r
    nc = bass.Bass("TRN2", target_bir_lowering=False, num_devices=NCORES)
    xg = nc.dram_tensor("xg", [E, C], f32r, kind="ExternalInput")
    nc.dram_tensor("w1", [E, HH], f32r, kind="ExternalInput")
    nc.dram_tensor("w2", [HH, E], f32r, kind="ExternalInput")
    nc.dram_tensor("bias", [HH + E], f32, kind="ExternalInput")
    y = nc.dram_tensor("y", [E, C], f32, kind="ExternalOutput")
    with _SplitDrainTC(nc) as tc:
        with tc.tile_pool(name="p", bufs=1) as p:
            t = p.tile([128, C], f32r, tag="t")
            nc.sync.dma_start(out=t[:], in_=xg.ap()[0:128, :])
            t2 = p.tile([128, C], f32, tag="t2")
            nc.vector.tensor_copy(t2[:], t[:])
            nc.sync.dma_start(out=y.ap()[0:128, :], in_=t2[:])
    return nc


def kernel(x, Wg, bg, W1, b1, W2, b2):
    x = np.asarray(x, dtype=np.float32)
    Wg = np.asarray(Wg, dtype=np.float32)
    bg = np.asarray(bg, dtype=np.float32)
    W1 = np.asarray(W1, dtype=np.float32)
    b1 = np.asarray(b1, dtype=np.float32)
    W2 = np.asarray(W2, dtype=np.float32)
    b2 = np.asarray(b2, dtype=np.float32)

    x2d = x.reshape(-1, E)  # [B*T, E]
    ntok = x2d.shape[0]

    # --- dispatch (host): gate + top-1 routing, gather per-expert tokens ---
    logits = x2d @ Wg + bg
    top = np.argmax(logits, axis=-1)
    idx = [np.nonzero(top == e)[0] for e in range(NEXP)]
    maxc = max(1, max(len(i) for i in idx))
    # capacity is a multiple of NSPLIT; capped so SBUF fits (per-partition
    # usage ~ 96*C + 76KB must stay under 192KB), with extra host-side
    # rounds for pathologically skewed routings.
    CCAP = 960
    C = min(((maxc + NSPLIT - 1) // NSPLIT) * NSPLIT, CCAP)
    rounds = (maxc + C - 1) // C

    zeros_b2 = np.zeros_like(b2[0])
    run = _get_runner(C)
    out = np.zeros((ntok, E), dtype=np.float32)
    for r in range(rounds):
        idx_r = [i[r * C : (r + 1) * C] for i in idx]
        in_maps = []
        for c in range(NCORES):
            e, hh = c // 2, c % 2
            xt = np.zeros((E, C), dtype=np.float32)
            xt[:, : len(idx_r[e])] = x2d[idx_r[e]].T
            in_maps.append(
                {
                    "xg": xt,
                    "w1": np.ascontiguousarray(W1[e][:, hh * HH : (hh + 1) * HH]),
                    "w2": np.ascontiguousarray(W2[e][hh * HH : (hh + 1) * HH, :]),
                    "bias": np.concatenate(
                        [
                            b1[e][hh * HH : (hh + 1) * HH],
                            b2[e] if hh == 0 else zeros_b2,
                        ]
                    ),
                }
            )
        results = run(in_maps)

        # --- combine (host): add hidden-half partials, scatter to tokens ---
        for e in range(NEXP):
            if len(idx_r[e]) == 0:
                continue
            ye = results[2 * e]["y"] + results[2 * e + 1]["y"]  # [E, C]
            out[idx_r[e]] = ye.T[: len(idx_r[e])]
    return out.reshape(B, T, E)


# revision 39
# speedup vs baseline: 1.0059x; 1.0059x over previous
"""MoE feed-forward (top-1 routing) on 8 TRN2 NeuronCores.

Sharding: expert-parallel with a tensor-parallel split of the hidden dim.
Core c handles expert e = c // 2 and hidden half hh = c % 2 (H=3072 -> 1536
per core).  The host computes the (tiny) gate + argmax routing, gathers each
expert's tokens into a fixed-capacity feature-major buffer (the dispatch /
all-to-all step of the sharding), and each core runs

    y_half = GELU(x_e @ W1[e][:, half] + b1[e][half]) @ W2[e][half, :] (+ b2[e])

entirely on device (f32r matmuls on the PE + Gelu on the ACT engine).  The
host combine adds the two hidden-half partial outputs of each expert pair and
scatters rows back to token positions.

Toolchain note: this walrus build accepts at most ONE sync-wait per
instruction.  The kernel is structured so Tile never needs more: <=8 DMAs
(no HW-queue sem reuse), "observer" ops that let PE/ACT see input DMAs once,
and a TileContext subclass that splits the final drain's waits.
"""

import sys

sys.path.insert(0, "/opt/trn_rl_repo")

import numpy as np

import concourse.bass as bass
import concourse.mybir as mybir
import concourse.tile as tile
from concourse import bass_utils
from concourse.vector_clock import ScopedClock

B, T, E, H, NEXP = 2, 1024, 768, 3072, 4
NCORES = 8
HH = H // 2          # hidden half per core: 1536
KE = E // 128        # 6   k-chunks over E
KH = HH // 128       # 12  k-chunks over HH
NSPLIT = 320         # matmul moving free-dim tile (>=256 keeps f32r at 1 cyc/row)

_MAXW = 1  # walrus allows a single sync-wait per instruction


class _SplitDrainTC(tile.TileContext):
    """TileContext whose final drain splits its sem waits across single-wait
    sync-engine event-sem instructions."""

    def _drain_and_barrier(self, tick_clock, wait_clock):
        carrier = self.nc.sync.nop(nofuse=True)
        wait_clock.add_sem_waits(
            carrier.ins, ScopedClock({None: tick_clock.global_clock})
        )
        waits = list(carrier.ins.sync_info.on_wait or [])
        if len(waits) > _MAXW:
            handles = {h.name: h for h in self.sems.allocated().values()}
            carrier.ins.sync_info.on_wait = waits[:_MAXW]
            for w in waits[_MAXW:]:
                self.nc.sync.wait_ge(handles[w.ant_name], w.wait_value)
        self.nc.sync.drain()
        self.nc.all_engine_barrier()
        popped = self.nc._tile_sem_poison_stack.pop()
        assert popped is self._sem_poison
        # The sem clear runs on the sync engine after the barrier; every other
        # engine's stream has already ended and the runtime serializes NEFF
        # executions, so the closing all-engine barrier is dead time and is
        # omitted.
        self.nc.clear_and_free_semaphores(list(self.sems.allocated().values()))


_prog_cache: dict[int, bass.Bass] = {}
_runner_cache: dict[int, object] = {}


class _Runner:
    """Compile once, execute many: replicates bass2jax.run_bass_via_pjrt but
    caches the jitted shard_map executable so repeat kernel() calls skip
    retracing, and exposes device-resident execution for timing."""

    def __init__(self, nc: bass.Bass):
        import jax
        from jax.sharding import Mesh, PartitionSpec, NamedSharding
        from jax.experimental.shard_map import shard_map
        from concourse import bass2jax

        bass2jax.install_neuronx_cc_hook()
        self.jax = jax
        partition_name = (
            nc.partition_id_tensor.name if nc.partition_id_tensor else None
        )
        in_names, out_names, out_avals, zero_outs = [], [], [], []
        for alloc in nc.m.functions[0].allocations:
            if not isinstance(alloc, mybir.MemoryLocationSet):
                continue
            name = alloc.memorylocations[0].name
            if alloc.kind == "ExternalInput":
                if name != partition_name:
                    in_names.append(name)
            elif alloc.kind == "ExternalOutput":
                shape = tuple(alloc.tensor_shape)
                dtype = mybir.dt.np(alloc.dtype)
                out_names.append(name)
                out_avals.append(jax.core.ShapedArray(shape, dtype))
                zero_outs.append(np.zeros(shape, dtype))
        self.in_names = list(in_names)
        self.out_names = out_names
        self.out_avals = out_avals
        self.zero_outs = zero_outs
        n_params = len(in_names)
        self.n_params = n_params
        all_in_names = list(in_names) + list(out_names)
        if partition_name is not None:
            all_in_names.append(partition_name)

        def _body(*args):
            operands = list(args)
            if partition_name is not None:
                operands.append(bass2jax.partition_id_tensor())
            outs = bass2jax._bass_exec_p.bind(
                *operands,
                out_avals=tuple(out_avals),
                in_names=tuple(all_in_names),
                out_names=tuple(out_names),
                lowering_input_output_aliases=(),
                sim_require_finite=True,
                sim_require_nnan=True,
                nc=nc,
            )
            return tuple(outs)

        devices = jax.devices()[:NCORES]
        self.mesh = Mesh(np.asarray(devices), ("core",))
        self.pspec = PartitionSpec("core")
        self.sharding = NamedSharding(self.mesh, self.pspec)
        n_outs = len(out_names)
        donate = tuple(range(n_params, n_params + n_outs))
        self.sharded = jax.jit(
            shard_map(
                _body,
                mesh=self.mesh,
                in_specs=(self.pspec,) * (n_params + n_outs),
                out_specs=(self.pspec,) * n_outs,
                check_rep=False,
            ),
            donate_argnums=donate,
            keep_unused=True,
        )

    def concat_inputs(self, in_maps):
        return [
            np.concatenate([np.asarray(m[name]) for m in in_maps], axis=0)
            for name in self.in_names
        ]

    def concat_zeros(self):
        return [
            np.zeros((NCORES * z.shape[0], *z.shape[1:]), z.dtype)
            for z in self.zero_outs
        ]

    def __call__(self, in_maps):
        out_arrs = self.sharded(*self.concat_inputs(in_maps), *self.concat_zeros())
        results = []
        for c in range(NCORES):
            results.append(
                {
                    name: np.asarray(out_arrs[i]).reshape(
                        NCORES, *self.out_avals[i].shape
                    )[c]
                    for i, name in enumerate(self.out_names)
                }
            )
        return results


def _get_runner(C: int) -> _Runner:
    r = _runner_cache.get(C)
    if r is None:
        nc = _prog_cache.get(C)
        if nc is None:
            nc = _build_program(C)
            _prog_cache[C] = nc
        r = _Runner(nc)
        _runner_cache[C] = r
    return r


def _build_program(C: int) -> bass.Bass:
    """One SPMD program: per-core expert-half MLP over C capacity tokens."""
    f32 = mybir.dt.float32
    f32r = mybir.dt.float32r
    nc = bass.Bass("TRN2", target_bir_lowering=False, num_devices=NCORES)

    xg = nc.dram_tensor("xg", [E, C], f32r, kind="ExternalInput")
    w1 = nc.dram_tensor("w1", [E, HH], f32r, kind="ExternalInput")
    w2 = nc.dram_tensor("w2", [HH, E], f32r, kind="ExternalInput")
    # bias pre-arranged by host to [128, KH+KE] so the DMA has contiguous
    # 72B rows (the 1-D "(m p) -> p m" view is 4B-element runs: 2304
    # descriptors at the 7ns floor ~= 1us at the head of the load window).
    bias = nc.dram_tensor("bias", [128, KH + KE], f32, kind="ExternalInput")
    y = nc.dram_tensor("y", [E, C], f32, kind="ExternalOutput")

    ncol = C // NSPLIT  # moving-dim tiles per row block

    with _SplitDrainTC(nc) as tc:
        with (
            tc.tile_pool(name="wpool", bufs=1) as wp,
            tc.tile_pool(name="ps", bufs=7, space="PSUM") as ps,
            tc.tile_pool(name="psW", bufs=1, space="PSUM") as psw,
        ):
            xgs = wp.tile([128, KE, C], f32r, tag="xg")
            w1s = wp.tile([128, KE, HH], f32r, tag="w1")
            w2s = wp.tile([128, KH, E], f32r, tag="w2")
            bs = wp.tile([128, KH + KE], f32, tag="bias")
            xgv = xg.ap().rearrange("(k p) c -> p k c", p=128)
            w1v = w1.ap().rearrange("(k p) h -> p k h", p=128)
            w2v = w2.ap().rearrange("(k p) e -> p k e", p=128)
            # Input DMAs: bias + xg first, W1 in per-k chunks (so the first
            # wave of phase A can start as each chunk lands), then W2
            # (streams under phase A).  10 input + 6 output DMAs = two full
            # rounds of the 8 HWDGE lanes.
            nc.sync.dma_start(out=bs[:], in_=bias.ap())
            nc.sync.dma_start(out=xgs[:], in_=xgv[:])
            for k in range(KE):
                nc.sync.dma_start(out=w1s[:, k, :], in_=w1v[:, k, :])
            nc.sync.dma_start(out=w2s[:, : KH // 2, :], in_=w2v[:, : KH // 2, :])
            nc.sync.dma_start(out=w2s[:, KH // 2 :, :], in_=w2v[:, KH // 2 :, :])

            hs = wp.tile([128, KH, C], f32r, tag="h")
            ys = wp.tile([128, KE, C], f32, tag="y")

            # This walrus build allows one sync-wait per instruction, so
            # "observer" ops let each engine see an input DMA exactly once;
            # Tile's per-engine clock then elides those waits elsewhere.
            warm = psw.tile([2, 32], f32, tag="warm")
            obs_i = [0]

            def pe_obs(src):
                i = obs_i[0]
                obs_i[0] += 1
                nc.tensor.matmul(
                    warm[:, 2 * i : 2 * i + 2], src, src, start=True, stop=True
                )

            pe_obs(xgs[:, 0, :2])
            scratch = wp.tile([128, KE + 3], f32, tag="actwarm")
            nc.scalar.activation(
                scratch[:, 0:1], bs[:, 0:1], mybir.ActivationFunctionType.Copy
            )
            nc.vector.tensor_copy(scratch[:, KE + 2 : KE + 3], bs[:, 0:1])

            # Phase A: h = gelu(x @ W1h + b1h), feature-major [HH, C].
            # Wave 1 = first 6 (m, n) groups run k-outer across 6 psum banks,
            # paced by the w1 chunk DMAs, so the PE starts ~7.5us in instead
            # of waiting for the whole W1 load.
            wave1 = [(m, n) for m in range(3) for n in range(ncol)] + [(3, 0)]
            accs = {}
            for g in wave1:
                acc = ps.tile([128, NSPLIT], f32, tag="acc")
                accs[g] = acc
            # No per-chunk observers: the first wave-1 matmul of each k-chunk
            # carries the w1_k DMA wait itself (its only unobserved dep).
            for k in range(KE):
                for m, n in wave1:
                    nc.tensor.matmul(
                        accs[(m, n)][:],
                        w1s[:, k, m * 128 : (m + 1) * 128],
                        xgs[:, k, n * NSPLIT : (n + 1) * NSPLIT],
                        start=(k == 0),
                        stop=(k == KE - 1),
                    )
            for m, n in wave1:
                nc.scalar.activation(
                    hs[:, m, n * NSPLIT : (n + 1) * NSPLIT],
                    accs[(m, n)][:],
                    mybir.ActivationFunctionType.Gelu,
                    bias=bs[:, m : m + 1],
                )
            rest = [
                (m, n)
                for m in range(3, KH)
                for n in range(ncol)
                if (m, n) not in wave1
            ]
            for m, n in rest:
                if True:
                    acc = ps.tile([128, NSPLIT], f32, tag="acc")
                    for k in range(KE):
                        nc.tensor.matmul(
                            acc[:],
                            w1s[:, k, m * 128 : (m + 1) * 128],
                            xgs[:, k, n * NSPLIT : (n + 1) * NSPLIT],
                            start=(k == 0),
                            stop=(k == KE - 1),
                        )
                    nc.scalar.activation(
                        hs[:, m, n * NSPLIT : (n + 1) * NSPLIT],
                        acc[:],
                        mybir.ActivationFunctionType.Gelu,
                        bias=bs[:, m : m + 1],
                    )

            # w2 observers between the phases: w2's DMA streams under phase A,
            # so gating only phase B on it costs nothing.  ACT observers of
            # one w1 chunk per HWDGE lane let the per-m output DMAs skip
            # their lane-FIFO waits.
            pe_obs(w2s[:, 0, :2])
            pe_obs(w2s[:, KH // 2, :2])
            for k in range(KE):
                nc.scalar.activation(
                    scratch[:, k + 1 : k + 2],
                    w1s[:, k, 0:1],
                    mybir.ActivationFunctionType.Copy,
                )
            nc.scalar.activation(
                scratch[:, KE + 1 : KE + 2],
                xgs[:, 0, 0:1],
                mybir.ActivationFunctionType.Copy,
            )

            # Phase B: y = h @ W2h + b2, feature-major [E, C]; per-m output
            # DMAs issued from the ACT engine's HWDGE (producer engine, so no
            # cross-engine data wait).
            yv = y.ap().rearrange("(m p) c -> p m c", p=128)
            for m in range(KE):
                for n in range(ncol):
                    acc = ps.tile([128, NSPLIT], f32, tag="acc")
                    for k in range(KH):
                        nc.tensor.matmul(
                            acc[:],
                            w2s[:, k, m * 128 : (m + 1) * 128],
                            hs[:, k, n * NSPLIT : (n + 1) * NSPLIT],
                            start=(k == 0),
                            stop=(k == KH - 1),
                        )
                    nc.vector.tensor_scalar_add(
                        ys[:, m, n * NSPLIT : (n + 1) * NSPLIT],
                        acc[:],
                        bs[:, KH + m : KH + m + 1],
                    )
                nc.scalar.dma_start(out=yv[:, m, :], in_=ys[:, m, :])

    return nc


def _build_null_program(C: int) -> bass.Bass:
    """Same I/O signature, near-zero device work — timing baseline only."""
    f32 = mybir.dt.float32
    f32r = mybir.dt.float32r
    nc = bass.Bass("TRN2", target_bir_lowering=False, num_devices=NCORES)
    xg = nc.dram_tensor("xg", [E, C], f32r, kind="ExternalInput")
    nc.dram_tensor("w1", [E, HH], f32r, kind="ExternalInput")
    nc.dram_tensor("w2", [HH, E], f32r, kind="ExternalInput")
    nc.dram_tensor("bias", [128, KH + KE], f32, kind="ExternalInput")
    y = nc.dram_tensor("y", [E, C], f32, kind="ExternalOutput")
    with _SplitDrainTC(nc) as tc:
        with tc.tile_pool(name="p", bufs=1) as p:
            t = p.tile([128, C], f32r, tag="t")
            nc.sync.dma_start(out=t[:], in_=xg.ap()[0:128, :])
            t2 = p.tile([128, C], f32, tag="t2")
            nc.vector.tensor_copy(t2[:], t[:])
            nc.sync.dma_start(out=y.ap()[0:128, :], in_=t2[:])
    return nc


def kernel(x, Wg, bg, W1, b1, W2, b2):
    x = np.asarray(x, dtype=np.float32)
    Wg = np.asarray(Wg, dtype=np.float32)
    bg = np.asarray(bg, dtype=np.float32)
    W1 = np.asarray(W1, dtype=np.float32)
    b1 = np.asarray(b1, dtype=np.float32)
    W2 = np.asarray(W2, dtype=np.float32)
    b2 = np.asarray(b2, dtype=np.float32)

    x2d = x.reshape(-1, E)  # [B*T, E]
    ntok = x2d.shape[0]

    # --- dispatch (host): gate + top-1 routing, gather per-expert tokens ---
    logits = x2d @ Wg + bg
    top = np.argmax(logits, axis=-1)
    idx = [np.nonzero(top == e)[0] for e in range(NEXP)]
    maxc = max(1, max(len(i) for i in idx))
    # capacity is a multiple of NSPLIT; capped so SBUF fits (per-partition
    # usage ~ 96*C + 76KB must stay under 192KB), with extra host-side
    # rounds for pathologically skewed routings.
    CCAP = 960
    C = min(((maxc + NSPLIT - 1) // NSPLIT) * NSPLIT, CCAP)
    rounds = (maxc + C - 1) // C

    zeros_b2 = np.zeros_like(b2[0])
    run = _get_runner(C)
    out = np.zeros((ntok, E), dtype=np.float32)
    for r in range(rounds):
        idx_r = [i[r * C : (r + 1) * C] for i in idx]
        in_maps = []
        for c in range(NCORES):
            e, hh = c // 2, c % 2
            xt = np.zeros((E, C), dtype=np.float32)
            xt[:, : len(idx_r[e])] = x2d[idx_r[e]].T
            in_maps.append(
                {
                    "xg": xt,
                    "w1": np.ascontiguousarray(W1[e][:, hh * HH : (hh + 1) * HH]),
                    "w2": np.ascontiguousarray(W2[e][hh * HH : (hh + 1) * HH, :]),
                    "bias": np.ascontiguousarray(
                        np.concatenate(
                            [
                                b1[e][hh * HH : (hh + 1) * HH],
                                b2[e] if hh == 0 else zeros_b2,
                            ]
                        ).reshape(KH + KE, 128).T
                    ),
                }
            )
        results = run(in_maps)

        # --- combine (host): add hidden-half partials, scatter to tokens ---
        for e in range(NEXP):
            if len(idx_r[e]) == 0:
                continue
            ye = results[2 * e]["y"] + results[2 * e + 1]["y"]  # [E, C]
            out[idx_r[e]] = ye.T[: len(idx_r[e])]
    return out.reshape(B, T, E)
